# revision 24
# baseline (speedup 1.0000x reference)
"""Causal self-attention with RoPE on 8 trn2 NeuronCores (~441us HW).

Sharding: tensor-parallel over heads (Megatron style). 16 heads, 8 cores
-> 2 heads per core. Each core computes q/k/v for its 2 heads, causal
attention, and a partial output projection against its w_o column slice.
Host sums the 8 partial outputs (the Megatron all-reduce, done at gather).

Device-side design (bf16 compute, fp32 PSUM accumulation throughout):
 - xT [C, B*T] bf16: x pre-transposed on host so the QKV contraction dim
   (c) is on SBUF partitions; no on-device transpose of x.
 - w_qk packed per head into even/odd-dim column blocks [QE|QO|KE|KO];
   QKV matmuls produce q/k directly in [d, t] layout, head-stacked so
   RoPE runs full-128-partition DVE tensor_tensor ops (6 per tile).
   PSUM is freed via one wide ScalarE copy per tile; rope reads SBUF.
 - v in natural [t, d] layout (same x tiles, shared j/c loop), stored
   with a ones column per head: AV then yields y[tq, 0:128] AND the
   softmax denominator y[:, 128] from a single accumulated matmul.
 - Post-rope q/k repacked (SBUF->SBUF DMA) into per-head [d, t] tensors
   so scores are single K=128 matmuls: scoresT[ts, tq] = khat.T @ qhat.
 - Softmax: no max-subtraction (scores ~N(0,1)); exp on ScalarE with the
   1/sqrt(D) scale folded in, emitted over paired 1024-wide tq blocks to
   amortize per-instruction overhead; causal masking only on
   diagonal-touching tiles via 0/1 bf16 mask multiply.
 - Normalize with reciprocal + per-partition tensor_scalar, PE-transpose
   y -> yT, output projection accumulates both heads in PSUM, ScalarE/DVE
   bounce to SBUF, DMA out as a partial [B*T, C] f32 result.

Compile recipe (this container): bacc.Bacc("TRN2") + TileContext +
nc.finalize() before run_bass_kernel_spmd (bacc legalizes multi-wait
instructions; raw bass.Bass fails walrus codegen here).
"""

import math

import numpy as np

B, T, C, H = 2, 2048, 2048, 16
D = C // H  # 128
NCORES = 8
HPC = H // NCORES  # heads per core = 2
N = B * T  # 4096 token rows
TT = T // 128  # 16 t-tiles per batch
NB = T // 512  # 4 n/tq blocks of 512 per batch
CT = C // 128  # 16 contraction tiles

_COMPILED = None


def _build():
    import concourse.bacc as bacc
    import concourse.mybir as mybir
    import concourse.tile as tile
    from concourse.masks import make_identity

    f32 = mybir.dt.float32
    bf16 = mybir.dt.bfloat16

    nc = bacc.Bacc("TRN2", target_bir_lowering=False, debug=False)
    xT = nc.declare_dram_parameter("xT", [C, N], bf16, isOutput=False)
    w_qk = nc.declare_dram_parameter("w_qk", [C, 4 * D], bf16, isOutput=False)
    w_v = nc.declare_dram_parameter("w_v", [C, HPC * D], bf16, isOutput=False)
    w_o = nc.declare_dram_parameter("w_o", [HPC * D, C], bf16, isOutput=False)
    cos2 = nc.declare_dram_parameter("cos2", [D, N], bf16, isOutput=False)
    sin2 = nc.declare_dram_parameter("sin2", [D, N], bf16, isOutput=False)
    # masks: 4 variants [128,1024] (diag in left half, right half ones)
    # then 4 variants [128,512] (diag within the single block)
    masks = nc.declare_dram_parameter("masks", [128, 4 * 1024 + 4 * 512], bf16, isOutput=False)
    out_p = nc.declare_dram_parameter("out_p", [N, C], f32, isOutput=True)

    SCALE = 1.0 / math.sqrt(D)
    VW = HPC * D + 2 * HPC  # 260: per t-tile v storage [v_h0|1|pad|v_h1|1|pad]

    with tile.TileContext(nc) as tc:
        with (
            tc.tile_pool(name="wpool", bufs=1) as wpool,
            tc.tile_pool(name="xpool", bufs=8) as xpool,
            tc.tile_pool(name="eo", bufs=4) as eopool,
            tc.tile_pool(name="ropetmp", bufs=6) as tmppool,
            tc.tile_pool(name="vsb", bufs=1) as vpool,
            tc.tile_pool(name="expp", bufs=18) as exppool,
            tc.tile_pool(name="ysb", bufs=4) as ypool,
            tc.tile_pool(name="rsb", bufs=8) as rpool,
            tc.tile_pool(name="yts", bufs=2) as ytpool,
            tc.tile_pool(name="pbig", bufs=2, space="PSUM") as pbig,
            tc.tile_pool(name="paux", bufs=4, space="PSUM") as paux,
        ):
            # ---- resident weights / constants ----
            wqk_sb = wpool.tile([128, CT * 512], bf16, tag="wqk")
            nc.sync.dma_start(
                out=wqk_sb[:, :].rearrange("p (kt e) -> p kt e", kt=CT),
                in_=w_qk.rearrange("(kt p) e -> p kt e", p=128),
            )
            wv_sb = wpool.tile([128, CT * 256], bf16, tag="wv")
            nc.sync.dma_start(
                out=wv_sb[:, :].rearrange("p (kt e) -> p kt e", kt=CT),
                in_=w_v.rearrange("(kt p) e -> p kt e", p=128),
            )
            wo_sb = wpool.tile([128, HPC * C], bf16, tag="wo")
            nc.sync.dma_start(
                out=wo_sb[:, :].rearrange("p (kt o) -> p kt o", kt=HPC),
                in_=w_o.rearrange("(kt p) o -> p kt o", p=128),
            )
            cos_sb = wpool.tile([128, N], bf16, tag="cos")
            nc.sync.dma_start(out=cos_sb[:, :], in_=cos2[:, :])
            sin_sb = wpool.tile([128, N], bf16, tag="sin")
            nc.sync.dma_start(out=sin_sb[:, :], in_=sin2[:, :])
            mask_sb = wpool.tile([128, 4 * 1024 + 4 * 512], bf16, tag="mask")
            nc.sync.dma_start(out=mask_sb[:, :], in_=masks[:, :])
            ident = wpool.tile([128, 128], bf16, tag="ident")
            make_identity(nc, ident[:, :])

            v_sb = vpool.tile([128, TT * VW], bf16, tag="vsb")
            for tt in range(TT):
                for h in range(HPC):
                    col = tt * VW + h * 130 + 128
                    nc.vector.memset(v_sb[:, col : col + 1], 1.0)

            for b in range(B):
                n0 = b * T

                # ---- phase QK: q,k projection in [d, t] layout + RoPE ----
                # Two 2-bank psum tiles per j: [QE | QO] and [KE | KO].
                qe2 = eopool.tile([128, T], bf16, tag="eo", name="qe2")
                qo2 = eopool.tile([128, T], bf16, tag="eo", name="qo2")
                ke2 = eopool.tile([128, T], bf16, tag="eo", name="ke2")
                ko2 = eopool.tile([128, T], bf16, tag="eo", name="ko2")
                rot = [(qe2, qo2), (ke2, ko2)]
                qhat = [eopool.tile([128, T], bf16, tag="qh", name=f"qhat{_h}") for _h in range(HPC)]
                khat = [eopool.tile([128, T], bf16, tag="qh", name=f"khat{_h}") for _h in range(HPC)]
                for j in range(NB):
                    js = slice(j * 512, (j + 1) * 512)
                    ps_q = pbig.tile([128, 1024], f32, tag="big", name="ps_q")
                    ps_k = pbig.tile([128, 1024], f32, tag="big", name="ps_k")
                    ps_v = [paux.tile([128, 256], f32, tag="aux", name=f"ps_v{_p}") for _p in range(4)]
                    for c in range(CT):
                        xt = xpool.tile([128, 512], bf16, tag="xt")
                        nc.gpsimd.dma_start(
                            out=xt[:, :],
                            in_=xT[c * 128 : (c + 1) * 128, n0 + j * 512 : n0 + (j + 1) * 512],
                        )
                        for part in range(4):  # QE, QO, KE, KO
                            dst = (ps_q, ps_q, ps_k, ps_k)[part]
                            off = (0, 512, 0, 512)[part]
                            wsl = wqk_sb[:, c * 512 + part * 128 : c * 512 + (part + 1) * 128]
                            nc.tensor.matmul(
                                dst[:, off : off + 512],
                                wsl,
                                xt[:, :],
                                start=(c == 0),
                                stop=(c == CT - 1),
                            )
                        for tl in range(4):
                            nc.tensor.matmul(
                                ps_v[tl][:, :],
                                xt[:, tl * 128 : (tl + 1) * 128],
                                wv_sb[:, c * 256 : (c + 1) * 256],
                                start=(c == 0),
                                stop=(c == CT - 1),
                            )
                    ce = cos_sb[:, n0 + j * 512 : n0 + (j + 1) * 512]
                    se = sin_sb[:, n0 + j * 512 : n0 + (j + 1) * 512]
                    # One wide ACT copy per psum tile frees the banks fast;
                    # rope then runs from SBUF off the PE critical path.
                    for qk in range(2):  # 0 = q, 1 = k
                        pc = tmppool.tile([128, 1024], f32, tag="rt", name=f"pc{qk}")
                        nc.scalar.copy(pc[:, :], (ps_q, ps_k)[qk][:, :])
                        E_sb, O_sb = pc[:, 0:512], pc[:, 512:1024]
                        dst_e, dst_o = rot[qk]
                        t1 = tmppool.tile([128, 512], f32, tag="rt2")
                        t2 = tmppool.tile([128, 512], f32, tag="rt2")
                        nc.vector.tensor_mul(t1[:, :], E_sb, ce)
                        nc.vector.tensor_mul(t2[:, :], O_sb, se)
                        nc.vector.tensor_sub(dst_e[:, js], t1[:, :], t2[:, :])
                        t3 = tmppool.tile([128, 512], f32, tag="rt2")
                        t4 = tmppool.tile([128, 512], f32, tag="rt2")
                        nc.vector.tensor_mul(t3[:, :], E_sb, se)
                        nc.vector.tensor_mul(t4[:, :], O_sb, ce)
                        nc.vector.tensor_add(dst_o[:, js], t3[:, :], t4[:, :])
                    for tl in range(4):
                        tt = j * 4 + tl
                        base = tt * VW
                        for h in range(HPC):
                            nc.vector.tensor_copy(
                                v_sb[:, base + h * 130 : base + h * 130 + 128],
                                ps_v[tl][:, h * 128 : (h + 1) * 128],
                            )
                    for h in range(HPC):
                        hb = 64 * h
                        nc.sync.dma_start(out=qhat[h][0:64, js], in_=qe2[hb : hb + 64, js])
                        nc.sync.dma_start(out=qhat[h][64:128, js], in_=qo2[hb : hb + 64, js])
                        nc.sync.dma_start(out=khat[h][0:64, js], in_=ke2[hb : hb + 64, js])
                        nc.sync.dma_start(out=khat[h][64:128, js], in_=ko2[hb : hb + 64, js])

                # ---- attention per head: paired tq blocks (jlo, jhi) share
                # one [128,1024] score psum + one wide exp instruction ----
                yT = [eopool.tile([128, T], bf16, tag="yt", name=f"yT{_h}") for _h in range(HPC)]
                for h in range(HPC):
                    for jp in range(NB // 2):
                        jlo, jhi = 2 * jp, 2 * jp + 1
                        exp_of = {}  # i -> (tile, base col of jlo half or None)
                        for i in range(4 * jhi + 4):
                            isl = slice(i * 128, (i + 1) * 128)
                            combined = i <= 4 * jlo + 3
                            sc = pbig.tile([128, 1024], f32, tag="big", name="sc")
                            ex = exppool.tile([128, 1024], bf16, tag="ex")
                            if combined:
                                nc.tensor.matmul(
                                    sc[:, 0:512], khat[h][:, isl],
                                    qhat[h][:, jlo * 512 : (jlo + 1) * 512],
                                    start=True, stop=True,
                                )
                                nc.tensor.matmul(
                                    sc[:, 512:1024], khat[h][:, isl],
                                    qhat[h][:, jhi * 512 : (jhi + 1) * 512],
                                    start=True, stop=True,
                                )
                                nc.scalar.activation(
                                    ex[:, :], sc[:, :],
                                    mybir.ActivationFunctionType.Exp, scale=SCALE,
                                )
                                p = i - 4 * jlo
                                if p >= 0:
                                    nc.vector.tensor_mul(
                                        ex[:, :], ex[:, :],
                                        mask_sb[:, p * 1024 : (p + 1) * 1024],
                                    )
                                exp_of[i] = (ex, 0)
                            else:
                                nc.tensor.matmul(
                                    sc[:, 0:512], khat[h][:, isl],
                                    qhat[h][:, jhi * 512 : (jhi + 1) * 512],
                                    start=True, stop=True,
                                )
                                nc.scalar.activation(
                                    ex[:, 0:512], sc[:, 0:512],
                                    mybir.ActivationFunctionType.Exp, scale=SCALE,
                                )
                                p = i - 4 * jhi
                                if p >= 0:
                                    nc.vector.tensor_mul(
                                        ex[:, 0:512], ex[:, 0:512],
                                        mask_sb[:, 4096 + p * 512 : 4096 + (p + 1) * 512],
                                    )
                                exp_of[i] = (ex, None)

                        for j in (jlo, jhi):
                            half = 0 if j == jlo else 512
                            y_ps = [paux.tile([128, 129], f32, tag="aux", name=f"y_ps{_p}") for _p in range(4)]
                            for tau in range(4):
                                g = 4 * j + tau
                                for i in range(g + 1):
                                    ex, base = exp_of[i]
                                    col = (half if base == 0 else 0) + tau * 128
                                    nc.tensor.matmul(
                                        y_ps[tau][:, :],
                                        ex[:, col : col + 128],
                                        v_sb[:, i * VW + h * 130 : i * VW + h * 130 + 129],
                                        start=(i == 0),
                                        stop=(i == g),
                                    )
                            for tau in range(4):
                                g = 4 * j + tau
                                r = rpool.tile([128, 1], f32, tag="r")
                                nc.vector.reciprocal(r[:, :], y_ps[tau][:, 128:129])
                                y_sb = ypool.tile([128, 128], bf16, tag="y")
                                nc.vector.tensor_scalar_mul(
                                    y_sb[:, :], y_ps[tau][:, 0:128], r[:, 0:1]
                                )
                                yt_ps = paux.tile([128, 128], bf16, tag="aux")
                                nc.tensor.transpose(yt_ps[:, :], y_sb[:, :], ident[:, :])
                                nc.vector.tensor_copy(
                                    yT[h][:, g * 128 : (g + 1) * 128], yt_ps[:, :]
                                )

                # ---- output projection (partial over this core's heads) ----
                for tt in range(TT):
                    tsl = slice(tt * 128, (tt + 1) * 128)
                    for obp in range(2):  # pairs of 512-wide o blocks
                        o_ps = pbig.tile([128, 1024], f32, tag="big", name="o_ps")
                        for ob in (2 * obp, 2 * obp + 1):
                            off = (ob - 2 * obp) * 512
                            for h in range(HPC):
                                nc.tensor.matmul(
                                    o_ps[:, off : off + 512],
                                    yT[h][:, tsl],
                                    wo_sb[:, h * C + ob * 512 : h * C + (ob + 1) * 512],
                                    start=(h == 0),
                                    stop=(h == HPC - 1),
                                )
                        yo = ytpool.tile([128, 1024], f32, tag="yo")
                        nc.vector.tensor_copy(yo[:, :], o_ps[:, :])
                        nc.sync.dma_start(
                            out=out_p[n0 + tt * 128 : n0 + (tt + 1) * 128, obp * 1024 : (obp + 1) * 1024],
                            in_=yo[:, :],
                        )
    nc.finalize()
    return nc


def _prep_inputs(x, w_qkv, w_o, rope_cos, rope_sin):
    import ml_dtypes

    bf = ml_dtypes.bfloat16
    xTh = np.ascontiguousarray(x.reshape(N, C).T).astype(bf)
    cosT = np.ascontiguousarray(rope_cos.T)  # [64, T]
    sinT = np.ascontiguousarray(rope_sin.T)
    cos2 = np.tile(np.concatenate([cosT, cosT], 0), (1, B)).astype(bf)
    sin2 = np.tile(np.concatenate([sinT, sinT], 0), (1, B)).astype(bf)

    r = np.arange(128)[:, None]
    c = np.arange(512)[None, :]
    singles = [((c - r) >= 128 * p).astype(np.float32) for p in range(4)]
    ones512 = np.ones((128, 512), dtype=np.float32)
    combos = [np.concatenate([s, ones512], 1) for s in singles]
    mk = np.concatenate(combos + singles, axis=1).astype(bf)

    ev = np.arange(0, D, 2)
    od = np.arange(1, D, 2)
    in_maps = []
    for m in range(NCORES):
        h0, h1 = 2 * m, 2 * m + 1
        # blocks QE|QO|KE|KO; within each, cols = [head0 dims | head1 dims]
        QE = np.concatenate([w_qkv[h0 * D + ev, :], w_qkv[h1 * D + ev, :]], 0).T
        QO = np.concatenate([w_qkv[h0 * D + od, :], w_qkv[h1 * D + od, :]], 0).T
        KE = np.concatenate([w_qkv[C + h0 * D + ev, :], w_qkv[C + h1 * D + ev, :]], 0).T
        KO = np.concatenate([w_qkv[C + h0 * D + od, :], w_qkv[C + h1 * D + od, :]], 0).T
        wqk_m = np.ascontiguousarray(np.concatenate([QE, QO, KE, KO], 1)).astype(bf)
        wv_m = np.ascontiguousarray(
            w_qkv[2 * C + 2 * m * D : 2 * C + (2 * m + 2) * D, :].T
        ).astype(bf)
        wo_m = np.ascontiguousarray(w_o[:, 2 * m * D : (2 * m + 2) * D].T).astype(bf)
        in_maps.append(
            {
                "xT": xTh,
                "w_qk": wqk_m,
                "w_v": wv_m,
                "w_o": wo_m,
                "cos2": cos2,
                "sin2": sin2,
                "masks": np.ascontiguousarray(mk),
            }
        )
    return in_maps


def kernel(x, w_qkv, w_o, rope_cos, rope_sin, _trace=False):
    global _COMPILED
    x = np.asarray(x, dtype=np.float32)
    w_qkv = np.asarray(w_qkv, dtype=np.float32)
    w_o = np.asarray(w_o, dtype=np.float32)
    rope_cos = np.asarray(rope_cos, dtype=np.float32)
    rope_sin = np.asarray(rope_sin, dtype=np.float32)

    from concourse.bass_utils import run_bass_kernel_spmd

    if _COMPILED is None:
        _COMPILED = _build()
    nc = _COMPILED
    in_maps = _prep_inputs(x, w_qkv, w_o, rope_cos, rope_sin)
    res = run_bass_kernel_spmd(
        nc, in_maps, core_ids=list(range(NCORES)), trace=_trace
    )
    out = np.zeros((N, C), dtype=np.float32)
    for m in range(NCORES):
        out += res.results[m]["out_p"]
    kernel._last_results = res
    return out.reshape(B, T, C)
